# revision 1
# baseline (speedup 1.0000x reference)
"""BinaryConv2D Trainium2 kernel.

Full computation:
  out = conv2d(sign(pad(x)), sign(k)) * avgpool3x3(mean|pad(x)|_ci) * alpha + bias

Device strategy (8 NeuronCores, data-parallel over batch N=32 -> 4 images/core):
  - Host binarizes x and k to exact +-1 bf16 and lays x out channel-major
    [n, ci, 58, 58] so the contraction dim (ci) lands on SBUF partitions.
  - The 3x3 conv = 9 shifted taps x 2 ci-chunks = 18 accumulating
    128x128x448 matmuls per PSUM tile (exact integer accumulation in f32).
  - Epilogue on DVE: psum * K[pix] then * alpha[co] + bias[co], written as
    out^T [co, pix] per image; host transposes back to NHWC.
"""

import sys

for _p in ("/root/.axon_site/_ro/trn_rl_repo", "/opt/trn_rl_repo"):
    if _p not in sys.path:
        sys.path.append(_p)

import numpy as np
import ml_dtypes

import concourse.bass as bass  # noqa: F401  (registers arch tables)
import concourse.mybir as mybir
import concourse.tile as tile
from concourse import bacc
from concourse.bass_utils import run_bass_kernel_spmd

BF16 = mybir.dt.bfloat16
F32 = mybir.dt.float32

NCORES = 8
N, H, W, C = 32, 56, 56, 256
HP, WP = H + 2, W + 2          # padded spatial
NPIX = HP * WP                  # 3364
OPIX = H * W                    # 3136
NIMG = N // NCORES              # images per core
NCHUNK = 2                      # ci chunks of 128
COCHUNK = 2                     # co chunks of 128
GROUPS = 7                      # output-row groups per image
GROWS = H // GROUPS             # 8 rows per group
NFREE = GROWS * W               # 448 pixels per matmul group

_NC = None


def _build_nc():
    nc = bacc.Bacc("TRN2", target_bir_lowering=False, debug=False)

    xb = nc.dram_tensor("xb", [NIMG, NCHUNK, 128, NPIX], BF16, kind="ExternalInput")
    wb = nc.dram_tensor("wb", [128, 9, NCHUNK, C], BF16, kind="ExternalInput")
    kb = nc.dram_tensor("kb", [NIMG, 128, OPIX], F32, kind="ExternalInput")
    ab = nc.dram_tensor("ab", [128, COCHUNK], F32, kind="ExternalInput")
    bb = nc.dram_tensor("bb", [128, COCHUNK], F32, kind="ExternalInput")
    ob = nc.dram_tensor("ob", [NIMG, COCHUNK, 128, OPIX], F32, kind="ExternalOutput")

    with tile.TileContext(nc) as tc:
        with (
            tc.tile_pool(name="wp", bufs=1) as wp,
            tc.tile_pool(name="xp", bufs=2) as xp,
            tc.tile_pool(name="kp", bufs=2) as kp,
            tc.tile_pool(name="op", bufs=4) as op,
            tc.tile_pool(name="ps", bufs=4, space="PSUM") as ps,
        ):
            w_sb = wp.tile([128, 9, NCHUNK, C], BF16)
            nc.sync.dma_start(w_sb[:], wb[:])
            a_sb = wp.tile([128, COCHUNK], F32)
            nc.sync.dma_start(a_sb[:], ab[:])
            b_sb = wp.tile([128, COCHUNK], F32)
            nc.sync.dma_start(b_sb[:], bb[:])

            for img in range(NIMG):
                x_sb = xp.tile([128, NCHUNK, HP, WP], BF16)
                nc.sync.dma_start(
                    x_sb[:], xb[img].rearrange("k p (h w) -> p k h w", h=HP)
                )
                k_sb = kp.tile([128, OPIX], F32)
                nc.sync.dma_start(k_sb[:], kb[img])

                for c in range(COCHUNK):
                    for g in range(GROUPS):
                        pt = ps.tile([128, NFREE], F32)
                        i = 0
                        for t in range(9):
                            dh, dw = t // 3, t % 3
                            for k in range(NCHUNK):
                                nc.tensor.matmul(
                                    pt[:],
                                    w_sb[:, t, k, c * 128 : (c + 1) * 128],
                                    x_sb[:, k, g * GROWS + dh : g * GROWS + dh + GROWS, dw : dw + W],
                                    start=(i == 0),
                                    stop=(i == 17),
                                )
                                i += 1
                        o_sb = op.tile([128, NFREE], F32)
                        nc.vector.tensor_tensor(
                            o_sb[:],
                            pt[:],
                            k_sb[:, g * NFREE : (g + 1) * NFREE],
                            mybir.AluOpType.mult,
                        )
                        nc.vector.tensor_scalar(
                            o_sb[:],
                            o_sb[:],
                            a_sb[:, c : c + 1],
                            b_sb[:, c : c + 1],
                            mybir.AluOpType.mult,
                            mybir.AluOpType.add,
                        )
                        nc.sync.dma_start(
                            ob[img, c, :, g * NFREE : (g + 1) * NFREE], o_sb[:]
                        )

    nc.compile()
    return nc


def get_nc():
    global _NC
    if _NC is None:
        _NC = _build_nc()
    return _NC


def prep_inputs(x, kernel, bias):
    """Host-side prep: binarize, pad, transpose; returns per-core in_maps."""
    xp = np.pad(x, ((0, 0), (1, 1), (1, 1), (0, 0)))
    binx = np.where(xp > 0, np.float32(1.0), np.float32(-1.0))
    binx_t = np.ascontiguousarray(binx.transpose(0, 3, 1, 2)).astype(
        ml_dtypes.bfloat16
    )  # (N, C, HP, WP)
    xb_all = binx_t.reshape(N, NCHUNK, 128, NPIX)

    beta = np.abs(xp).mean(axis=3)  # (N, HP, WP) f32
    ks = beta[:, 0:H, :] + beta[:, 1 : H + 1, :] + beta[:, 2 : H + 2, :]
    K = (ks[:, :, 0:W] + ks[:, :, 1 : W + 1] + ks[:, :, 2 : W + 2]) / np.float32(9.0)
    K_flat = K.reshape(N, 1, OPIX).astype(np.float32)

    bink = np.where(kernel > 0, np.float32(1.0), np.float32(-1.0))
    wb = np.ascontiguousarray(
        bink.reshape(9, NCHUNK, 128, C).transpose(2, 0, 1, 3)
    ).astype(ml_dtypes.bfloat16)  # (128, 9, 2, 256)

    alpha = np.abs(kernel).mean(axis=(0, 1, 2)).astype(np.float32)  # (256,)
    ab = np.ascontiguousarray(alpha.reshape(COCHUNK, 128).T)  # (128, 2)
    bb = np.ascontiguousarray(bias.astype(np.float32).reshape(COCHUNK, 128).T)

    in_maps = []
    for core in range(NCORES):
        sl = slice(core * NIMG, (core + 1) * NIMG)
        in_maps.append(
            {
                "xb": np.ascontiguousarray(xb_all[sl]),
                "kb": np.ascontiguousarray(
                    np.broadcast_to(K_flat[sl], (NIMG, 128, OPIX))
                ),
                "wb": wb,
                "ab": ab,
                "bb": bb,
            }
        )
    return in_maps


def assemble_output(results):
    """results: list of 8 dicts with 'ob' (NIMG, 2, 128, OPIX) -> (N,H,W,C) f32."""
    ot = np.concatenate([r["ob"] for r in results], axis=0)  # (N, 2, 128, OPIX)
    out = ot.reshape(N, C, H, W).transpose(0, 2, 3, 1)
    return np.ascontiguousarray(out)


def kernel(x, kernel, bias, _trace=False):
    nc = get_nc()
    in_maps = prep_inputs(x, kernel, bias)
    res = run_bass_kernel_spmd(
        nc, in_maps, core_ids=list(range(NCORES)), trace=_trace
    )
    out = assemble_output(res.results)
    if _trace:
        return out, res
    return out


# revision 32
# speedup vs baseline: 2.1367x; 2.1367x over previous
"""BinaryConv2D Trainium2 kernel.

Full computation:
  out = conv2d(sign(pad(x)), sign(k)) * avgpool3x3(mean|pad(x)|_ci) * alpha + bias

Device strategy (8 NeuronCores, data-parallel over batch N=32 -> 4 images/core):
  - Host binarizes x and k to exact +-1 (bf16 or fp8e4m3 -- both represent +-1
    exactly) and lays x out channel-major [n, ci, 58*58pad] so the contraction
    dim (ci) lands on SBUF partitions.
  - The 3x3 conv = 9 shifted taps accumulated into PSUM (exact integer
    accumulation in f32).  fp8 DoubleRow contracts 256 ci per matmul,
    bf16 contracts 128 (x2 chunks).
  - Epilogue on DVE: psum * K[pix], then * alpha[co] + bias[co]; output is
    written as out^T [co, pix] per image; host transposes back to NHWC.
"""

import os
import sys

for _p in ("/root/.axon_site/_ro/trn_rl_repo", "/opt/trn_rl_repo"):
    if _p not in sys.path:
        sys.path.append(_p)

import numpy as np
import ml_dtypes

import concourse.bass as bass  # noqa: F401  (registers arch tables)
import concourse.mybir as mybir
import concourse.tile as tile
from concourse import bacc
from concourse.bass_utils import run_bass_kernel_spmd

BF16 = mybir.dt.bfloat16
FP8 = mybir.dt.float8e4
F32 = mybir.dt.float32

# MODE: "bf16" (contraction 128/mm) or "fp8dr" (DoubleRow, contraction 256/mm)
MODE = os.environ.get("CONV_MODE", "fp8dr")
# RHS: "flat" (contiguous 464 over padded 58-grid, garbage cols masked in
# epilogue) or "strided" (8x56 nested AP, valid pixels only)
RHS = os.environ.get("CONV_RHS", "flat")

NCORES = 8
N, H, W, C = 32, 56, 56, 256
HP, WP = H + 2, W + 2          # padded spatial 58x58
NPIX = HP * WP                  # 3364
XFREE = 3376                    # padded flat x free size (mult of 16, >= 3366)
OPIX = H * W                    # 3136
NIMG = N // NCORES              # images per core
GROUPS = 7                      # output-row groups per image
GROWS = H // GROUPS             # 8 rows per group
NVALID = GROWS * W              # 448 valid pixels per group
NFLAT = GROWS * WP              # 464 pixels incl garbage cols (flat rhs)
GSPAN = (GROWS + 2) * WP + 2    # 582: input span a group's 9 taps touch
# x split into 3 pieces per image (group 0 / groups 1-3 / groups 4-6) so the
# first matmuls only wait on a 149KB transfer
P0_LEN = GSPAN                  # 582: flat [0, 582)
P0_FREE = 592
PA_OFF = NFLAT                  # 464: flat start of piece A (groups 1-3)
PA_LEN = 3 * NFLAT + GSPAN - PA_OFF  # 1510: flat [464, 1974)
PB_OFF = 4 * NFLAT              # 1856: flat start of piece B
PB_LEN = NPIX + 2 - PB_OFF      # 1510: flat [1856, 3366)
PFREE = 1520                    # piece tile free size (mult of 16)

_NC = None


def _x_dtype():
    return FP8 if MODE == "fp8dr" else BF16


def _build_nc():
    nc = bacc.Bacc("TRN2", target_bir_lowering=False, debug=False)
    XD = _x_dtype()

    xb = nc.dram_tensor("xb", [NIMG, 2, 2, 128, PFREE], XD, kind="ExternalInput")
    x0 = nc.dram_tensor("x0", [NIMG, 2, 128, P0_FREE], XD, kind="ExternalInput")
    wb = nc.dram_tensor("wb", [128, 9, 2, C], XD, kind="ExternalInput")
    kb = nc.dram_tensor("kb", [NIMG, 1, OPIX], F32, kind="ExternalInput")
    ab = nc.dram_tensor("ab", [128, 2], F32, kind="ExternalInput")
    bb = nc.dram_tensor("bb", [128, 2], F32, kind="ExternalInput")
    ob = nc.dram_tensor("ob", [NIMG, 2, 128, OPIX], F32, kind="ExternalOutput")

    assert MODE == "fp8dr" and RHS == "flat", "optimized build supports fp8dr/flat"

    with tile.TileContext(nc) as tc:
        with (
            tc.tile_pool(name="wp", bufs=1) as wp,
            tc.tile_pool(name="xp", bufs=4) as xp,
            tc.tile_pool(name="kp", bufs=2) as kp,
            tc.tile_pool(name="op", bufs=4) as op,
            tc.tile_pool(name="ps", bufs=6, space="PSUM") as ps,
        ):
            # DMA queues drain FIFO round-robin, so issue order ~= completion
            # order.  Put the first matmul's exact dependencies (tap-0
            # weights, image-0 group-0 x) at the head of the pipe.
            def dma_w(t):
                wt = wp.tile([128, 2, C], XD, tag=f"w{t}")
                nc.sync.dma_start(wt[:], wb[:, t])
                return wt

            def dma_x0(img):
                x_0 = xp.tile([128, 2, P0_FREE], XD, tag="x0")
                nc.sync.dma_start(
                    x_0[:, :, :P0_LEN],
                    x0[img, :, :, :P0_LEN].rearrange("k p f -> p k f"),
                )
                return x_0

            def dma_xa(img):
                x_a = xp.tile([128, 2, PFREE], XD, tag="xa")
                nc.sync.dma_start(
                    x_a[:, :, :PA_LEN],
                    xb[img, 0, :, :, :PA_LEN].rearrange("k p f -> p k f"),
                )
                return x_a

            def dma_xb(img):
                x_b = xp.tile([128, 2, PFREE], XD, tag="xb")
                nc.sync.dma_start(
                    x_b[:, :, :PB_LEN],
                    xb[img, 1, :, :, :PB_LEN].rearrange("k p f -> p k f"),
                )
                return x_b

            def dma_k(img):
                k1_sb = kp.tile([1, OPIX], F32, tag="k1")
                nc.sync.dma_start(k1_sb[:], kb[img])
                k_sb = kp.tile([128, OPIX], F32, tag="kbig")
                nc.gpsimd.partition_broadcast(k_sb[:], k1_sb[:])
                return k_sb

            w_t = [None] * 9
            w_t[0] = dma_w(0)
            img0_x0 = dma_x0(0)
            for t in range(1, 9):
                w_t[t] = dma_w(t)
            img0_xa = dma_xa(0)
            img0_xb = dma_xb(0)
            img0_k = dma_k(0)
            a_sb = wp.tile([128, 2], F32, tag="a")
            nc.sync.dma_start(a_sb[:], ab[:])
            b_sb = wp.tile([128, 2], F32, tag="b")
            nc.sync.dma_start(b_sb[:], bb[:])

            # warm the PE clock (HAM) with throwaway matmuls on the tap-0
            # weight tile while x is still streaming in
            warm_ps = ps.tile([128, NFLAT], F32, tag="pt")
            warm_rhs = w_t[0].rearrange("p k c -> p (k c)")[:, :512].rearrange(
                "p (two f) -> p two f", two=2
            )
            for _ in range(5):
                nc.tensor.matmul(
                    warm_ps[:, :256],
                    w_t[0][:, :, 0:128],
                    warm_rhs,
                    start=True,
                    stop=True,
                    perf_mode=mybir.MatmulPerfMode.DoubleRow,
                )

            for img in range(NIMG):
                if img == 0:
                    x_0, x_a, x_b, k_sb = img0_x0, img0_xa, img0_xb, img0_k
                else:
                    x_0 = dma_x0(img)
                    x_a = dma_xa(img)
                    x_b = dma_xb(img)
                    k_sb = dma_k(img)

                for c in range(2):
                    for g in range(GROUPS):
                        pt = ps.tile([128, NFLAT], F32)
                        for t in range(9):
                            dh, dw = t // 3, t % 3
                            if g == 0:
                                src, off = x_0, dh * WP + dw
                            elif g < 4:
                                src, off = (
                                    x_a,
                                    g * NFLAT + dh * WP + dw - PA_OFF,
                                )
                            else:
                                src, off = x_b, g * NFLAT + dh * WP + dw - PB_OFF
                            nc.tensor.matmul(
                                pt[:],
                                w_t[t][:, :, c * 128 : (c + 1) * 128],
                                src[:, :, off : off + NFLAT],
                                start=(t == 0),
                                stop=(t == 8),
                                perf_mode=mybir.MatmulPerfMode.DoubleRow,
                            )

                        o_sb = op.tile([128, GROWS, W], F32)
                        ksl = k_sb[:, g * NVALID : (g + 1) * NVALID].rearrange(
                            "p (h w) -> p h w", w=W
                        )
                        pt_v = pt.rearrange("p (h w) -> p h w", w=WP)[:, :, 0:W]
                        nc.vector.tensor_tensor(
                            o_sb[:], pt_v, ksl, mybir.AluOpType.mult
                        )
                        nc.vector.tensor_scalar(
                            o_sb[:],
                            o_sb[:],
                            a_sb[:, c : c + 1],
                            b_sb[:, c : c + 1],
                            mybir.AluOpType.mult,
                            mybir.AluOpType.add,
                        )
                        nc.sync.dma_start(
                            ob[img, c, :, g * NVALID : (g + 1) * NVALID],
                            o_sb[:].rearrange("p h w -> p (h w)"),
                        )

    nc.compile()
    return nc


def get_nc():
    global _NC
    if _NC is None:
        _NC = _build_nc()
    return _NC


def prep_inputs(x, kernel, bias):
    """Host-side prep: binarize, pad, transpose; returns per-core in_maps."""
    np_xd = mybir.dt.np(_x_dtype())
    xp = np.pad(x, ((0, 0), (1, 1), (1, 1), (0, 0)))
    binx = np.where(xp > 0, np.float32(1.0), np.float32(-1.0))
    binx_t = np.ascontiguousarray(binx.transpose(0, 3, 1, 2)).astype(np_xd)
    flat = binx_t.reshape(N, 2, 128, NPIX)
    x0_all = np.zeros((N, 2, 128, P0_FREE), dtype=np_xd)
    x0_all[:, :, :, :P0_LEN] = flat[:, :, :, :P0_LEN]
    xb_all = np.zeros((N, 2, 2, 128, PFREE), dtype=np_xd)
    xb_all[:, 0, :, :, :PA_LEN] = flat[:, :, :, PA_OFF : PA_OFF + PA_LEN]
    xb_all[:, 1, :, :, : NPIX - PB_OFF] = flat[:, :, :, PB_OFF:]
    # piece B needs 2 elements past NPIX (garbage cols of the last row) --
    # they stay zero, matching the reference's zero padding semantics anyway

    beta = np.abs(xp).mean(axis=3)  # (N, HP, WP) f32
    ks = beta[:, 0:H, :] + beta[:, 1 : H + 1, :] + beta[:, 2 : H + 2, :]
    K = (ks[:, :, 0:W] + ks[:, :, 1 : W + 1] + ks[:, :, 2 : W + 2]) / np.float32(9.0)
    K_flat = np.ascontiguousarray(K.reshape(N, 1, OPIX).astype(np.float32))

    bink = np.where(kernel > 0, np.float32(1.0), np.float32(-1.0))
    wb = np.ascontiguousarray(
        bink.reshape(9, 2, 128, C).transpose(2, 0, 1, 3)
    ).astype(np_xd)  # (128, 9, 2, 256)

    alpha = np.abs(kernel).mean(axis=(0, 1, 2)).astype(np.float32)  # (256,)
    ab = np.ascontiguousarray(alpha.reshape(2, 128).T)  # (128, 2)
    bb = np.ascontiguousarray(bias.astype(np.float32).reshape(2, 128).T)

    in_maps = []
    for core in range(NCORES):
        sl = slice(core * NIMG, (core + 1) * NIMG)
        in_maps.append(
            {
                "xb": np.ascontiguousarray(xb_all[sl]),
                "x0": np.ascontiguousarray(x0_all[sl]),
                "kb": K_flat[sl],
                "wb": wb,
                "ab": ab,
                "bb": bb,
            }
        )
    return in_maps


def assemble_output(results):
    """results: list of 8 dicts with 'ob' (NIMG, 2, 128, OPIX) -> (N,H,W,C) f32."""
    ot = np.concatenate([r["ob"] for r in results], axis=0)  # (N, 2, 128, OPIX)
    out = ot.reshape(N, C, H, W).transpose(0, 2, 3, 1)
    return np.ascontiguousarray(out)


def kernel(x, kernel, bias, _trace=False):
    nc = get_nc()
    in_maps = prep_inputs(x, kernel, bias)
    res = run_bass_kernel_spmd(
        nc, in_maps, core_ids=list(range(NCORES)), trace=_trace
    )
    out = assemble_output(res.results)
    if _trace:
        return out, res
    return out


# revision 37
# speedup vs baseline: 2.1379x; 1.0006x over previous
"""BinaryConv2D Trainium2 kernel.

Full computation:
  out = conv2d(sign(pad(x)), sign(k)) * avgpool3x3(mean|pad(x)|_ci) * alpha + bias

Device strategy (8 NeuronCores, data-parallel over batch N=32 -> 4 images/core):
  - Host binarizes x and k to exact +-1 (bf16 or fp8e4m3 -- both represent +-1
    exactly) and lays x out channel-major [n, ci, 58*58pad] so the contraction
    dim (ci) lands on SBUF partitions.
  - The 3x3 conv = 9 shifted taps accumulated into PSUM (exact integer
    accumulation in f32).  fp8 DoubleRow contracts 256 ci per matmul,
    bf16 contracts 128 (x2 chunks).
  - Epilogue on DVE: psum * K[pix], then * alpha[co] + bias[co]; output is
    written as out^T [co, pix] per image; host transposes back to NHWC.
"""

import os
import sys

for _p in ("/root/.axon_site/_ro/trn_rl_repo", "/opt/trn_rl_repo"):
    if _p not in sys.path:
        sys.path.append(_p)

import numpy as np
import ml_dtypes

import concourse.bass as bass  # noqa: F401  (registers arch tables)
import concourse.mybir as mybir
import concourse.tile as tile
from concourse import bacc
from concourse.bass_utils import run_bass_kernel_spmd

BF16 = mybir.dt.bfloat16
FP8 = mybir.dt.float8e4
F32 = mybir.dt.float32

# MODE: "bf16" (contraction 128/mm) or "fp8dr" (DoubleRow, contraction 256/mm)
MODE = os.environ.get("CONV_MODE", "fp8dr")
# RHS: "flat" (contiguous 464 over padded 58-grid, garbage cols masked in
# epilogue) or "strided" (8x56 nested AP, valid pixels only)
RHS = os.environ.get("CONV_RHS", "flat")

NCORES = 8
N, H, W, C = 32, 56, 56, 256
HP, WP = H + 2, W + 2          # padded spatial 58x58
NPIX = HP * WP                  # 3364
XFREE = 3376                    # padded flat x free size (mult of 16, >= 3366)
OPIX = H * W                    # 3136
NIMG = N // NCORES              # images per core
GROUPS = 7                      # output-row groups per image
GROWS = H // GROUPS             # 8 rows per group
NVALID = GROWS * W              # 448 valid pixels per group
NFLAT = GROWS * WP              # 464 pixels incl garbage cols (flat rhs)
GSPAN = (GROWS + 2) * WP + 2    # 582: input span a group's 9 taps touch
# x split into 3 pieces per image (group 0 / groups 1-3 / groups 4-6) so the
# first matmuls only wait on a 149KB transfer
P0_LEN = GSPAN                  # 582: flat [0, 582)
P0_FREE = 592
PA_OFF = NFLAT                  # 464: flat start of piece A (groups 1-3)
PA_LEN = 3 * NFLAT + GSPAN - PA_OFF  # 1510: flat [464, 1974)
PB_OFF = 4 * NFLAT              # 1856: flat start of piece B
PB_LEN = NPIX + 2 - PB_OFF      # 1510: flat [1856, 3366)
PFREE = 1520                    # piece tile free size (mult of 16)

_NC = None


def _x_dtype():
    return FP8 if MODE == "fp8dr" else BF16


def _build_nc():
    nc = bacc.Bacc("TRN2", target_bir_lowering=False, debug=False)
    XD = _x_dtype()

    xb = nc.dram_tensor("xb", [NIMG, 2, 2, 128, PFREE], XD, kind="ExternalInput")
    x0 = nc.dram_tensor("x0", [NIMG, 2, 128, P0_FREE], XD, kind="ExternalInput")
    wb = nc.dram_tensor("wb", [128, 9, 2, C], XD, kind="ExternalInput")
    kb = nc.dram_tensor("kb", [NIMG, 1, OPIX], F32, kind="ExternalInput")
    ab = nc.dram_tensor("ab", [128, 2], F32, kind="ExternalInput")
    bb = nc.dram_tensor("bb", [128, 2], F32, kind="ExternalInput")
    ob = nc.dram_tensor("ob", [NIMG, 2, 128, OPIX], F32, kind="ExternalOutput")

    assert MODE == "fp8dr" and RHS == "flat", "optimized build supports fp8dr/flat"

    with tile.TileContext(nc) as tc:
        with (
            tc.tile_pool(name="wp", bufs=1) as wp,
            tc.tile_pool(name="xp", bufs=4) as xp,
            tc.tile_pool(name="kp", bufs=2) as kp,
            tc.tile_pool(name="op", bufs=4) as op,
            tc.tile_pool(name="ps", bufs=6, space="PSUM") as ps,
        ):
            # DMA queues drain FIFO round-robin, so issue order ~= completion
            # order.  Put the first matmul's exact dependencies (tap-0
            # weights, image-0 group-0 x) at the head of the pipe.

            def dma_x0(img):
                x_0 = xp.tile([128, 2, P0_FREE], XD, tag="x0")
                nc.sync.dma_start(
                    x_0[:, :, :P0_LEN],
                    x0[img, :, :, :P0_LEN].rearrange("k p f -> p k f"),
                )
                return x_0

            def dma_xa(img):
                x_a = xp.tile([128, 2, PFREE], XD, tag="xa")
                nc.sync.dma_start(
                    x_a[:, :, :PA_LEN],
                    xb[img, 0, :, :, :PA_LEN].rearrange("k p f -> p k f"),
                )
                return x_a

            def dma_xb(img):
                x_b = xp.tile([128, 2, PFREE], XD, tag="xb")
                nc.sync.dma_start(
                    x_b[:, :, :PB_LEN],
                    xb[img, 1, :, :, :PB_LEN].rearrange("k p f -> p k f"),
                )
                return x_b

            def dma_k(img):
                k1_sb = kp.tile([1, OPIX], F32, tag="k1")
                nc.sync.dma_start(k1_sb[:], kb[img])
                k_sb = kp.tile([128, OPIX], F32, tag="kbig")
                nc.gpsimd.partition_broadcast(k_sb[:], k1_sb[:])
                return k_sb

            w_sb = wp.tile([128, 9, 2, C], XD)
            nc.sync.dma_start(w_sb[:], wb[:])
            a_sb = wp.tile([128, 2], F32, tag="a")
            nc.sync.dma_start(a_sb[:], ab[:])
            b_sb = wp.tile([128, 2], F32, tag="b")
            nc.sync.dma_start(b_sb[:], bb[:])

            # warm the PE clock (HAM) with throwaway matmuls on the tap-0
            # weight tile while x is still streaming in
            warm_ps = ps.tile([128, NFLAT], F32, tag="pt")
            warm_rhs = w_sb.rearrange("p t k c -> p (t k c)")[:, : 2 * NFLAT].rearrange(
                "p (two f) -> p two f", two=2
            )
            for _ in range(8):
                nc.tensor.matmul(
                    warm_ps[:],
                    w_sb[:, 0, :, 0:128],
                    warm_rhs,
                    start=True,
                    stop=True,
                    perf_mode=mybir.MatmulPerfMode.DoubleRow,
                )

            for img in range(NIMG):
                x_0 = dma_x0(img)
                x_a = dma_xa(img)
                x_b = dma_xb(img)
                k_sb = dma_k(img)

                for c in range(2):
                    for g in range(GROUPS):
                        pt = ps.tile([128, NFLAT], F32)
                        for t in range(9):
                            dh, dw = t // 3, t % 3
                            if g == 0:
                                src, off = x_0, dh * WP + dw
                            elif g < 4:
                                src, off = (
                                    x_a,
                                    g * NFLAT + dh * WP + dw - PA_OFF,
                                )
                            else:
                                src, off = x_b, g * NFLAT + dh * WP + dw - PB_OFF
                            nc.tensor.matmul(
                                pt[:],
                                w_sb[:, t, :, c * 128 : (c + 1) * 128],
                                src[:, :, off : off + NFLAT],
                                start=(t == 0),
                                stop=(t == 8),
                                perf_mode=mybir.MatmulPerfMode.DoubleRow,
                            )

                        o_sb = op.tile([128, GROWS, W], F32)
                        ksl = k_sb[:, g * NVALID : (g + 1) * NVALID].rearrange(
                            "p (h w) -> p h w", w=W
                        )
                        pt_v = pt.rearrange("p (h w) -> p h w", w=WP)[:, :, 0:W]
                        nc.vector.tensor_tensor(
                            o_sb[:], pt_v, ksl, mybir.AluOpType.mult
                        )
                        nc.vector.tensor_scalar(
                            o_sb[:],
                            o_sb[:],
                            a_sb[:, c : c + 1],
                            b_sb[:, c : c + 1],
                            mybir.AluOpType.mult,
                            mybir.AluOpType.add,
                        )
                        nc.sync.dma_start(
                            ob[img, c, :, g * NVALID : (g + 1) * NVALID],
                            o_sb[:].rearrange("p h w -> p (h w)"),
                        )

    nc.compile()
    return nc


def get_nc():
    global _NC
    if _NC is None:
        _NC = _build_nc()
    return _NC


def prep_inputs(x, kernel, bias):
    """Host-side prep: binarize, pad, transpose; returns per-core in_maps."""
    np_xd = mybir.dt.np(_x_dtype())
    xp = np.pad(x, ((0, 0), (1, 1), (1, 1), (0, 0)))
    binx = np.where(xp > 0, np.float32(1.0), np.float32(-1.0))
    binx_t = np.ascontiguousarray(binx.transpose(0, 3, 1, 2)).astype(np_xd)
    flat = binx_t.reshape(N, 2, 128, NPIX)
    x0_all = np.zeros((N, 2, 128, P0_FREE), dtype=np_xd)
    x0_all[:, :, :, :P0_LEN] = flat[:, :, :, :P0_LEN]
    xb_all = np.zeros((N, 2, 2, 128, PFREE), dtype=np_xd)
    xb_all[:, 0, :, :, :PA_LEN] = flat[:, :, :, PA_OFF : PA_OFF + PA_LEN]
    xb_all[:, 1, :, :, : NPIX - PB_OFF] = flat[:, :, :, PB_OFF:]
    # piece B needs 2 elements past NPIX (garbage cols of the last row) --
    # they stay zero, matching the reference's zero padding semantics anyway

    beta = np.abs(xp).mean(axis=3)  # (N, HP, WP) f32
    ks = beta[:, 0:H, :] + beta[:, 1 : H + 1, :] + beta[:, 2 : H + 2, :]
    K = (ks[:, :, 0:W] + ks[:, :, 1 : W + 1] + ks[:, :, 2 : W + 2]) / np.float32(9.0)
    K_flat = np.ascontiguousarray(K.reshape(N, 1, OPIX).astype(np.float32))

    bink = np.where(kernel > 0, np.float32(1.0), np.float32(-1.0))
    wb = np.ascontiguousarray(
        bink.reshape(9, 2, 128, C).transpose(2, 0, 1, 3)
    ).astype(np_xd)  # (128, 9, 2, 256)

    alpha = np.abs(kernel).mean(axis=(0, 1, 2)).astype(np.float32)  # (256,)
    ab = np.ascontiguousarray(alpha.reshape(2, 128).T)  # (128, 2)
    bb = np.ascontiguousarray(bias.astype(np.float32).reshape(2, 128).T)

    in_maps = []
    for core in range(NCORES):
        sl = slice(core * NIMG, (core + 1) * NIMG)
        in_maps.append(
            {
                "xb": np.ascontiguousarray(xb_all[sl]),
                "x0": np.ascontiguousarray(x0_all[sl]),
                "kb": K_flat[sl],
                "wb": wb,
                "ab": ab,
                "bb": bb,
            }
        )
    return in_maps


def assemble_output(results):
    """results: list of 8 dicts with 'ob' (NIMG, 2, 128, OPIX) -> (N,H,W,C) f32."""
    ot = np.concatenate([r["ob"] for r in results], axis=0)  # (N, 2, 128, OPIX)
    out = ot.reshape(N, C, H, W).transpose(0, 2, 3, 1)
    return np.ascontiguousarray(out)


def kernel(x, kernel, bias, _trace=False):
    nc = get_nc()
    in_maps = prep_inputs(x, kernel, bias)
    res = run_bass_kernel_spmd(
        nc, in_maps, core_ids=list(range(NCORES)), trace=_trace
    )
    out = assemble_output(res.results)
    if _trace:
        return out, res
    return out
